# revision 1
# baseline (speedup 1.0000x reference)
"""Bass/Trainium2 kernel for nn_LowRankLoss.

Reference computation:
  m      = mean(feat, axis=1)                      # [n, h, w], channel mean
  normed = m / ||m||_F (per sample)
  rank   = #(singular values of normed > 0)        # [n]
  loss   = sum(max(0, -(rank1 - rank2))) / n

The memory-bound part (target_regime=memory) is the channel-mean reduction
over two [128, 256, 32, 64] f32 tensors (512 MiB total). That runs on 8
NeuronCores, data-parallel over the batch dim (16 samples/core). The device
returns per-sample channel sums [n, 2048]; the tiny per-sample SVDs
(128 matrices of 32x64) and the scalar loss are finished on host.

Device layout per core (per input tensor x = [16, 2, 128, 2048]):
  - for each sample s and channel-half cb: DMA x[s, cb] -> SBUF [128, 2048]
    (1 MiB, fully contiguous in DRAM).
  - TensorE reduces over the channel (partition) dim:
      acc[:, s*16+j] (+)= tile[:, j*128:(j+1)*128].T @ ones[128, 1]
    i.e. stationary = data chunk, moving = ones; out partition = spatial%128.
    start/stop pairs accumulate the two channel halves in PSUM.
  - acc [128, 256] (one PSUM bank) -> SBUF via VectorE copy -> DMA out.
Host decodes acc[p, s*16+j] = sum_c x[s, c, j*128+p].
"""

import numpy as np

N_CORES = 8
NS = 16           # samples per core
C = 256           # channels
CB = 2            # channel blocks of 128
P = 128           # partitions
H, W = 32, 64
HW = H * W        # 2048 spatial
JCH = HW // P     # 16 spatial chunks of 128

_CACHE = {}


def _build_nc():
    import concourse.bacc as bacc
    import concourse.mybir as mybir
    import concourse.tile as tile

    nc = bacc.Bacc(None, target_bir_lowering=False)
    f32 = mybir.dt.float32

    x_raw = nc.dram_tensor("x_raw", [NS, CB, P, HW], f32, kind="ExternalInput")
    x_rect = nc.dram_tensor("x_rect", [NS, CB, P, HW], f32, kind="ExternalInput")
    out_raw = nc.dram_tensor("out_raw", [P, NS * JCH], f32, kind="ExternalOutput")
    out_rect = nc.dram_tensor("out_rect", [P, NS * JCH], f32, kind="ExternalOutput")

    with tile.TileContext(nc) as tc:
        with (
            tc.tile_pool(name="io", bufs=4) as pool,
            tc.tile_pool(name="small", bufs=2) as small,
            tc.tile_pool(name="psum", bufs=1, space="PSUM") as psum,
        ):
            ones = small.tile([P, 1], f32, tag="ones")
            nc.gpsimd.memset(ones[:], 1.0)

            for xt, ot, tg in ((x_raw, out_raw, "raw"), (x_rect, out_rect, "rect")):
                acc = psum.tile([P, NS * JCH], f32, tag=f"acc_{tg}")
                for s in range(NS):
                    t0 = pool.tile([P, HW], f32, tag="in0")
                    t1 = pool.tile([P, HW], f32, tag="in1")
                    nc.sync.dma_start(t0[:], xt[s, 0])
                    nc.sync.dma_start(t1[:], xt[s, 1])
                    for j in range(JCH):
                        col = s * JCH + j
                        nc.tensor.matmul(
                            acc[:, col : col + 1],
                            t0[:, j * P : (j + 1) * P],
                            ones[:],
                            start=True,
                            stop=False,
                        )
                        nc.tensor.matmul(
                            acc[:, col : col + 1],
                            t1[:, j * P : (j + 1) * P],
                            ones[:],
                            start=False,
                            stop=True,
                        )
                osb = small.tile([P, NS * JCH], f32, tag="osb")
                nc.vector.tensor_copy(osb[:], acc[:])
                nc.sync.dma_start(ot[:], osb[:])

    nc.compile()
    return nc


def _device_channel_sums(raw, rect, trace=False):
    """Run the bass kernel on 8 cores; return (sums_raw, sums_rect) [128, 2048]
    and the BassKernelResults."""
    from concourse.bass_utils import run_bass_kernel_spmd

    if "nc" not in _CACHE:
        _CACHE["nc"] = _build_nc()
    nc = _CACHE["nc"]

    raw5 = raw.reshape(N_CORES, NS, CB, P, HW)
    rect5 = rect.reshape(N_CORES, NS, CB, P, HW)
    in_maps = [{"x_raw": raw5[i], "x_rect": rect5[i]} for i in range(N_CORES)]
    res = run_bass_kernel_spmd(nc, in_maps, list(range(N_CORES)), trace=trace)

    def decode(arr):
        # arr[p, s*JCH + j] = sum_c x[s, c, j*128 + p] -> [NS, HW]
        return arr.reshape(P, NS, JCH).transpose(1, 2, 0).reshape(NS, HW)

    sums_raw = np.concatenate([decode(res.results[i]["out_raw"]) for i in range(N_CORES)])
    sums_rect = np.concatenate([decode(res.results[i]["out_rect"]) for i in range(N_CORES)])
    return sums_raw, sums_rect, res


def _rank_from_sums(sums):
    # channel mean (exact: /256 is a power of two), normalize, svd, count
    m = (sums / np.float32(C)).astype(np.float32)
    nrm = np.linalg.norm(m, axis=1, keepdims=True)
    normed = (m / nrm).reshape(-1, H, W)
    s = np.linalg.svd(normed.astype(np.float32), compute_uv=False)
    return (s > 0.0).sum(axis=1).astype(np.float32)


def kernel(raw_feat, rectified_feat, trace=False):
    raw = np.ascontiguousarray(np.asarray(raw_feat, dtype=np.float32))
    rect = np.ascontiguousarray(np.asarray(rectified_feat, dtype=np.float32))

    sums_raw, sums_rect, res = _device_channel_sums(raw, rect, trace=trace)
    _CACHE["last_results"] = res
    _CACHE["last_sums"] = (sums_raw, sums_rect)

    rank1 = _rank_from_sums(sums_raw)
    rank2 = _rank_from_sums(sums_rect)
    loss = np.maximum(np.float32(0.0), -(rank1 - rank2))
    loss = loss.sum(dtype=np.float32) / np.float32(raw.shape[0])
    return np.asarray(loss, dtype=np.float32)


# revision 6
# speedup vs baseline: 1.1924x; 1.1924x over previous
"""Bass/Trainium2 kernel for nn_LowRankLoss.

Reference computation:
  m      = mean(feat, axis=1)                      # [n, h, w], channel mean
  normed = m / ||m||_F (per sample)
  rank   = #(singular values of normed > 0)        # [n]
  loss   = sum(max(0, -(rank1 - rank2))) / n

The memory-bound part (target_regime=memory) is the channel-mean reduction
over two [128, 256, 32, 64] f32 tensors (512 MiB total). That runs on 8
NeuronCores, data-parallel over the batch dim (16 samples/core). The device
returns per-sample channel sums [n, 2048]; the tiny per-sample SVDs
(128 matrices of 32x64) and the scalar loss are finished on host.

Device design per core (per input tensor, viewed [G=2, SG=8, CG=16, CL=16, F=2048]):
  - Tile = 8 samples x 16 channels on partitions (p = s*16 + c), spatial on
    free dim. DMA is 128 runs of 8 KiB (strided, DMA-friendly).
  - One constant block-diagonal stationary S[128, 8] (S[p, m] = p//16 == m)
    turns TensorE into a per-sample channel reducer:
      acc[s, f] += sum_c tile[s*16+c, f]
    Data streams as the fp32r moving operand (1 cycle/row at N=512 vs 4 for
    fp32), N=512 per matmul, 16 channel-group matmuls accumulate in PSUM.
  - acc [8, 2048] (4 PSUM banks) -> SBUF via VectorE -> 64 KiB DMA out.
fp32r truncates the multiplier mantissa (~1e-3 rel err worst case), far below
what could flip a singular-value-positivity count (min sigma ~2e-2 here).
"""

import numpy as np

N_CORES = 8
NS = 16           # samples per core
C = 256           # channels
H, W = 32, 64
F = H * W         # 2048 spatial
G = 2             # sample groups per core
SG = 8            # samples per group (SG*CL = 128 partitions)
CL = 16           # channels per group-tile
CG = C // CL      # 16 channel groups
NB = 4            # matmuls per tile (N=512 PSUM bank limit)
BN = F // NB      # 512

_CACHE = {}


def _build_nc():
    import concourse.bacc as bacc
    import concourse.mybir as mybir
    import concourse.tile as tile

    nc = bacc.Bacc(None, target_bir_lowering=False)
    f32 = mybir.dt.float32
    f32r = mybir.dt.float32r

    x_raw = nc.dram_tensor("x_raw", [G, SG, CG, CL, F], f32, kind="ExternalInput")
    x_rect = nc.dram_tensor("x_rect", [G, SG, CG, CL, F], f32, kind="ExternalInput")
    out_raw = nc.dram_tensor("out_raw", [G, SG, F], f32, kind="ExternalOutput")
    out_rect = nc.dram_tensor("out_rect", [G, SG, F], f32, kind="ExternalOutput")

    with tile.TileContext(nc) as tc:
        with (
            tc.tile_pool(name="io", bufs=7) as pool,
            tc.tile_pool(name="red", bufs=3) as redp,
            tc.tile_pool(name="small", bufs=2) as small,
            tc.tile_pool(name="psum", bufs=2, space="PSUM") as psum,
        ):
            s_np = np.zeros((SG * CL, SG), np.float32)
            for m in range(SG):
                s_np[m * CL : (m + 1) * CL, m] = 1.0
            s_dram = nc.inline_tensor(s_np, name="s_const")
            s_stage = small.tile([SG * CL, SG], f32, tag="stat_stage")
            nc.sync.dma_start(s_stage[:], s_dram[:])
            S = small.tile([SG * CL, SG], f32r, tag="stat")
            nc.vector.tensor_copy(S[:], s_stage[:])

            CGH = CG // 2  # channel-group pairs folded by the DVE add
            for xt, ot in ((x_raw, out_raw), (x_rect, out_rect)):
                for g in range(G):
                    acc = psum.tile([SG, F], f32, tag="acc")
                    for cg in range(CGH):
                        t0 = pool.tile([SG * CL, F], f32, tag="in0")
                        t1 = pool.tile([SG * CL, F], f32, tag="in1")
                        nc.sync.dma_start(t0[:], xt[g, :, cg])
                        nc.sync.dma_start(t1[:], xt[g, :, cg + CGH])
                        # pairwise channel fold + round to fp32r for the PE
                        tr = redp.tile([SG * CL, F], f32r, tag="red")
                        nc.vector.tensor_add(tr[:], t0[:], t1[:])
                        for j in range(NB):
                            nc.tensor.matmul(
                                acc[:, j * BN : (j + 1) * BN],
                                S[:],
                                tr[:, j * BN : (j + 1) * BN],
                                start=(cg == 0),
                                stop=(cg == CGH - 1),
                            )
                    osb = small.tile([SG, F], f32, tag="osb")
                    nc.vector.tensor_copy(osb[:], acc[:])
                    nc.sync.dma_start(ot[g], osb[:])

    nc.compile()
    return nc


def _device_channel_sums(raw, rect, trace=False):
    """Run the bass kernel on 8 cores; return (sums_raw, sums_rect) [128, 2048]
    and the BassKernelResults."""
    from concourse.bass_utils import run_bass_kernel_spmd

    if "nc" not in _CACHE:
        _CACHE["nc"] = _build_nc()
    nc = _CACHE["nc"]

    raw6 = raw.reshape(N_CORES, G, SG, CG, CL, F)
    rect6 = rect.reshape(N_CORES, G, SG, CG, CL, F)
    in_maps = [{"x_raw": raw6[i], "x_rect": rect6[i]} for i in range(N_CORES)]
    res = run_bass_kernel_spmd(nc, in_maps, list(range(N_CORES)), trace=trace)

    sums_raw = np.concatenate(
        [res.results[i]["out_raw"].reshape(NS, F) for i in range(N_CORES)]
    )
    sums_rect = np.concatenate(
        [res.results[i]["out_rect"].reshape(NS, F) for i in range(N_CORES)]
    )
    return sums_raw, sums_rect, res


def _rank_from_sums(sums):
    # channel mean (exact: /256 is a power of two), normalize, svd, count
    m = (sums / np.float32(C)).astype(np.float32)
    nrm = np.linalg.norm(m, axis=1, keepdims=True)
    normed = (m / nrm).reshape(-1, H, W)
    s = np.linalg.svd(normed.astype(np.float32), compute_uv=False)
    return (s > 0.0).sum(axis=1).astype(np.float32)


def kernel(raw_feat, rectified_feat, trace=False):
    raw = np.ascontiguousarray(np.asarray(raw_feat, dtype=np.float32))
    rect = np.ascontiguousarray(np.asarray(rectified_feat, dtype=np.float32))

    sums_raw, sums_rect, res = _device_channel_sums(raw, rect, trace=trace)
    _CACHE["last_results"] = res
    _CACHE["last_sums"] = (sums_raw, sums_rect)

    rank1 = _rank_from_sums(sums_raw)
    rank2 = _rank_from_sums(sums_rect)
    loss = np.maximum(np.float32(0.0), -(rank1 - rank2))
    loss = loss.sum(dtype=np.float32) / np.float32(raw.shape[0])
    return np.asarray(loss, dtype=np.float32)


# revision 10
# speedup vs baseline: 1.2134x; 1.0176x over previous
"""Bass/Trainium2 kernel for nn_LowRankLoss.

Reference computation:
  m      = mean(feat, axis=1)                      # [n, h, w], channel mean
  normed = m / ||m||_F (per sample)
  rank   = #(singular values of normed > 0)        # [n]
  loss   = sum(max(0, -(rank1 - rank2))) / n

The memory-bound part (target_regime=memory) is the channel-mean reduction
over two [128, 256, 32, 64] f32 tensors (512 MiB total). That runs on 8
NeuronCores, data-parallel over the batch dim (16 samples/core). The device
returns per-sample channel sums [n, 2048]; the tiny per-sample SVDs
(128 matrices of 32x64) and the scalar loss are finished on host.

Device design per core (per input tensor, viewed [G=2, SG=8, CG=16, CL=16, F=2048]):
  - Tile = 8 samples x 16 channels on partitions (p = s*16 + c), spatial on
    free dim. DMA is 128 runs of 8 KiB (strided, DMA-friendly).
  - One constant block-diagonal stationary S[128, 8] (S[p, m] = p//16 == m)
    turns TensorE into a per-sample channel reducer:
      acc[s, f] += sum_c tile[s*16+c, f]
    Data streams as the fp32r moving operand (1 cycle/row at N=512 vs 4 for
    fp32), N=512 per matmul, 16 channel-group matmuls accumulate in PSUM.
  - acc [8, 2048] (4 PSUM banks) -> SBUF via VectorE -> 64 KiB DMA out.
fp32r truncates the multiplier mantissa (~1e-3 rel err worst case), far below
what could flip a singular-value-positivity count (min sigma ~2e-2 here).
"""

import numpy as np

N_CORES = 8
NS = 16           # samples per core
C = 256           # channels
H, W = 32, 64
F = H * W         # 2048 spatial
G = 2             # sample groups per core
SG = 8            # samples per group (SG*CL = 128 partitions)
CL = 16           # channels per group-tile
CG = C // CL      # 16 channel groups
NB = 4            # matmuls per tile (N=512 PSUM bank limit)
BN = F // NB      # 512

_CACHE = {}


def _build_nc():
    import concourse.bacc as bacc
    import concourse.mybir as mybir
    import concourse.tile as tile

    nc = bacc.Bacc(None, target_bir_lowering=False)
    f32 = mybir.dt.float32
    f32r = mybir.dt.float32r

    x_raw = nc.dram_tensor("x_raw", [G, SG, CG, CL, F], f32, kind="ExternalInput")
    x_rect = nc.dram_tensor("x_rect", [G, SG, CG, CL, F], f32, kind="ExternalInput")
    out_raw = nc.dram_tensor("out_raw", [G, SG, F], f32, kind="ExternalOutput")
    out_rect = nc.dram_tensor("out_rect", [G, SG, F], f32, kind="ExternalOutput")

    with tile.TileContext(nc) as tc:
        with (
            tc.tile_pool(name="io", bufs=6) as pool,
            tc.tile_pool(name="red", bufs=3) as redp,
            tc.tile_pool(name="small", bufs=2) as small,
            tc.tile_pool(name="psum", bufs=2, space="PSUM") as psum,
        ):
            s_np = np.zeros((SG * CL, SG), np.float32)
            for m in range(SG):
                s_np[m * CL : (m + 1) * CL, m] = 1.0
            s_dram = nc.inline_tensor(s_np, name="s_const")
            s_stage = small.tile([SG * CL, SG], f32, tag="stat_stage")
            nc.sync.dma_start(s_stage[:], s_dram[:])
            S = small.tile([SG * CL, SG], f32r, tag="stat")
            nc.vector.tensor_copy(S[:], s_stage[:])

            CGH = CG // 2  # channel-group pairs folded by the DVE add
            dma_engines = (nc.sync, nc.scalar, nc.gpsimd)
            dma_i = 0
            for xt, ot in ((x_raw, out_raw), (x_rect, out_rect)):
                for g in range(G):
                    acc = psum.tile([SG, F], f32, tag="acc")
                    for cg in range(CGH):
                        t0 = pool.tile([SG * CL, F], f32, tag="in0")
                        t1 = pool.tile([SG * CL, F], f32, tag="in1")
                        e0 = dma_engines[dma_i % 3]
                        e1 = dma_engines[(dma_i + 1) % 3]
                        dma_i += 2
                        e0.dma_start(t0[:], xt[g, :, cg])
                        e1.dma_start(t1[:], xt[g, :, cg + CGH])
                        # pairwise channel fold + round to fp32r for the PE
                        tr = redp.tile([SG * CL, F], f32r, tag="red")
                        nc.vector.tensor_add(tr[:], t0[:], t1[:])
                        for j in range(NB):
                            nc.tensor.matmul(
                                acc[:, j * BN : (j + 1) * BN],
                                S[:],
                                tr[:, j * BN : (j + 1) * BN],
                                start=(cg == 0),
                                stop=(cg == CGH - 1),
                            )
                    osb = small.tile([SG, F], f32, tag="osb")
                    nc.vector.tensor_copy(osb[:], acc[:])
                    nc.sync.dma_start(ot[g], osb[:])

    nc.compile()
    return nc


def _device_channel_sums(raw, rect, trace=False):
    """Run the bass kernel on 8 cores; return (sums_raw, sums_rect) [128, 2048]
    and the BassKernelResults."""
    from concourse.bass_utils import run_bass_kernel_spmd

    if "nc" not in _CACHE:
        _CACHE["nc"] = _build_nc()
    nc = _CACHE["nc"]

    raw6 = raw.reshape(N_CORES, G, SG, CG, CL, F)
    rect6 = rect.reshape(N_CORES, G, SG, CG, CL, F)
    in_maps = [{"x_raw": raw6[i], "x_rect": rect6[i]} for i in range(N_CORES)]
    res = run_bass_kernel_spmd(nc, in_maps, list(range(N_CORES)), trace=trace)

    sums_raw = np.concatenate(
        [res.results[i]["out_raw"].reshape(NS, F) for i in range(N_CORES)]
    )
    sums_rect = np.concatenate(
        [res.results[i]["out_rect"].reshape(NS, F) for i in range(N_CORES)]
    )
    return sums_raw, sums_rect, res


def _rank_from_sums(sums):
    # channel mean (exact: /256 is a power of two), normalize, svd, count
    m = (sums / np.float32(C)).astype(np.float32)
    nrm = np.linalg.norm(m, axis=1, keepdims=True)
    normed = (m / nrm).reshape(-1, H, W)
    s = np.linalg.svd(normed.astype(np.float32), compute_uv=False)
    return (s > 0.0).sum(axis=1).astype(np.float32)


def kernel(raw_feat, rectified_feat, trace=False):
    raw = np.ascontiguousarray(np.asarray(raw_feat, dtype=np.float32))
    rect = np.ascontiguousarray(np.asarray(rectified_feat, dtype=np.float32))

    sums_raw, sums_rect, res = _device_channel_sums(raw, rect, trace=trace)
    _CACHE["last_results"] = res
    _CACHE["last_sums"] = (sums_raw, sums_rect)

    rank1 = _rank_from_sums(sums_raw)
    rank2 = _rank_from_sums(sums_rect)
    loss = np.maximum(np.float32(0.0), -(rank1 - rank2))
    loss = loss.sum(dtype=np.float32) / np.float32(raw.shape[0])
    return np.asarray(loss, dtype=np.float32)


# revision 11
# speedup vs baseline: 1.3003x; 1.0716x over previous
"""Bass/Trainium2 kernel for nn_LowRankLoss.

Reference computation:
  m      = mean(feat, axis=1)                      # [n, h, w], channel mean
  normed = m / ||m||_F (per sample)
  rank   = #(singular values of normed > 0)        # [n]
  loss   = sum(max(0, -(rank1 - rank2))) / n

The memory-bound part (target_regime=memory) is the channel-mean reduction
over two [128, 256, 32, 64] f32 tensors (512 MiB total). That runs on 8
NeuronCores, data-parallel over the batch dim (16 samples/core). The device
returns per-sample channel sums [n, 2048]; the tiny per-sample SVDs
(128 matrices of 32x64) and the scalar loss are finished on host.

Device design per core (per input tensor, viewed [G=2, SG=8, CG=16, CL=16, F=2048]):
  - Tile = 8 samples x 16 channels on partitions (p = s*16 + c), spatial on
    free dim. DMA is 128 runs of 8 KiB (strided, DMA-friendly).
  - One constant block-diagonal stationary S[128, 8] (S[p, m] = p//16 == m)
    turns TensorE into a per-sample channel reducer:
      acc[s, f] += sum_c tile[s*16+c, f]
    Data streams as the fp32r moving operand (1 cycle/row at N=512 vs 4 for
    fp32), N=512 per matmul, 16 channel-group matmuls accumulate in PSUM.
  - acc [8, 2048] (4 PSUM banks) -> SBUF via VectorE -> 64 KiB DMA out.
fp32r truncates the multiplier mantissa (~1e-3 rel err worst case), far below
what could flip a singular-value-positivity count (min sigma ~2e-2 here).
"""

import numpy as np

N_CORES = 8
NS = 16           # samples per core
C = 256           # channels
H, W = 32, 64
F = H * W         # 2048 spatial
G = 2             # sample groups per core
SG = 8            # samples per group (SG*CL = 128 partitions)
CL = 16           # channels per group-tile
CG = C // CL      # 16 channel groups
NB = 4            # matmuls per tile (N=512 PSUM bank limit)
BN = F // NB      # 512

_CACHE = {}


def _build_nc():
    import concourse.bacc as bacc
    import concourse.mybir as mybir
    import concourse.tile as tile

    nc = bacc.Bacc(None, target_bir_lowering=False)
    f32 = mybir.dt.float32
    f32r = mybir.dt.float32r

    x_raw = nc.dram_tensor("x_raw", [G, SG, CG, CL, F], f32, kind="ExternalInput")
    x_rect = nc.dram_tensor("x_rect", [G, SG, CG, CL, F], f32, kind="ExternalInput")
    out_raw = nc.dram_tensor("out_raw", [G, SG, F], f32, kind="ExternalOutput")
    out_rect = nc.dram_tensor("out_rect", [G, SG, F], f32, kind="ExternalOutput")

    with tile.TileContext(nc) as tc:
        with (
            tc.tile_pool(name="io", bufs=6) as pool,
            tc.tile_pool(name="red", bufs=3) as redp,
            tc.tile_pool(name="small", bufs=2) as small,
            tc.tile_pool(name="psum", bufs=2, space="PSUM") as psum,
        ):
            s_np = np.zeros((SG * CL, SG), np.float32)
            for m in range(SG):
                s_np[m * CL : (m + 1) * CL, m] = 1.0
            s_dram = nc.inline_tensor(s_np, name="s_const")
            s_stage = small.tile([SG * CL, SG], f32, tag="stat_stage")
            nc.sync.dma_start(s_stage[:], s_dram[:])
            S = small.tile([SG * CL, SG], f32r, tag="stat")
            nc.vector.tensor_copy(S[:], s_stage[:])

            CGH = CG // 2  # channel-group pairs folded by the DVE add
            dma_engines = (nc.sync, nc.scalar, nc.gpsimd)
            dma_i = 0
            for xt, ot in ((x_raw, out_raw), (x_rect, out_rect)):
                for g in range(G):
                    acc = psum.tile([SG, F], f32, tag="acc")
                    for cg in range(CGH):
                        t0 = pool.tile([SG * CL, F], f32, tag="in0")
                        t1 = pool.tile([SG * CL, F], f32, tag="in1")
                        # SWDGE (gpsimd) spreads each DMA across all 16 SDMA
                        # engines; HWDGE rings only reach 8 of them.
                        nc.gpsimd.dma_start(t0[:], xt[g, :, cg])
                        nc.gpsimd.dma_start(t1[:], xt[g, :, cg + CGH])
                        # pairwise channel fold + round to fp32r for the PE
                        tr = redp.tile([SG * CL, F], f32r, tag="red")
                        nc.vector.tensor_add(tr[:], t0[:], t1[:])
                        for j in range(NB):
                            nc.tensor.matmul(
                                acc[:, j * BN : (j + 1) * BN],
                                S[:],
                                tr[:, j * BN : (j + 1) * BN],
                                start=(cg == 0),
                                stop=(cg == CGH - 1),
                            )
                    osb = small.tile([SG, F], f32, tag="osb")
                    nc.vector.tensor_copy(osb[:], acc[:])
                    nc.sync.dma_start(ot[g], osb[:])

    nc.compile()
    return nc


def _device_channel_sums(raw, rect, trace=False):
    """Run the bass kernel on 8 cores; return (sums_raw, sums_rect) [128, 2048]
    and the BassKernelResults."""
    from concourse.bass_utils import run_bass_kernel_spmd

    if "nc" not in _CACHE:
        _CACHE["nc"] = _build_nc()
    nc = _CACHE["nc"]

    raw6 = raw.reshape(N_CORES, G, SG, CG, CL, F)
    rect6 = rect.reshape(N_CORES, G, SG, CG, CL, F)
    in_maps = [{"x_raw": raw6[i], "x_rect": rect6[i]} for i in range(N_CORES)]
    res = run_bass_kernel_spmd(nc, in_maps, list(range(N_CORES)), trace=trace)

    sums_raw = np.concatenate(
        [res.results[i]["out_raw"].reshape(NS, F) for i in range(N_CORES)]
    )
    sums_rect = np.concatenate(
        [res.results[i]["out_rect"].reshape(NS, F) for i in range(N_CORES)]
    )
    return sums_raw, sums_rect, res


def _rank_from_sums(sums):
    # channel mean (exact: /256 is a power of two), normalize, svd, count
    m = (sums / np.float32(C)).astype(np.float32)
    nrm = np.linalg.norm(m, axis=1, keepdims=True)
    normed = (m / nrm).reshape(-1, H, W)
    s = np.linalg.svd(normed.astype(np.float32), compute_uv=False)
    return (s > 0.0).sum(axis=1).astype(np.float32)


def kernel(raw_feat, rectified_feat, trace=False):
    raw = np.ascontiguousarray(np.asarray(raw_feat, dtype=np.float32))
    rect = np.ascontiguousarray(np.asarray(rectified_feat, dtype=np.float32))

    sums_raw, sums_rect, res = _device_channel_sums(raw, rect, trace=trace)
    _CACHE["last_results"] = res
    _CACHE["last_sums"] = (sums_raw, sums_rect)

    rank1 = _rank_from_sums(sums_raw)
    rank2 = _rank_from_sums(sums_rect)
    loss = np.maximum(np.float32(0.0), -(rank1 - rank2))
    loss = loss.sum(dtype=np.float32) / np.float32(raw.shape[0])
    return np.asarray(loss, dtype=np.float32)


# revision 13
# speedup vs baseline: 2.5295x; 1.9452x over previous
"""Bass/Trainium2 kernel for nn_LowRankLoss.

Reference computation:
  m      = mean(feat, axis=1)                      # [n, h, w], channel mean
  normed = m / ||m||_F (per sample)
  rank   = #(singular values of normed > 0)        # [n]
  loss   = sum(max(0, -(rank1 - rank2))) / n

The memory-bound part (target_regime=memory) is the channel-mean reduction
over two [128, 256, 32, 64] f32 tensors (512 MiB total). That runs on 8
NeuronCores, data-parallel over the batch dim (16 samples/core). The device
returns per-sample channel sums [n, 2048]; the tiny per-sample SVDs
(128 matrices of 32x64) and the scalar loss are finished on host.

Device design per core (per input tensor, viewed [NS=16, 2, 128, F=2048]):
  - One fully contiguous 2 MiB DMA per sample -> SBUF [128, 4096]
    (channel half cb in free cols [cb*F, (cb+1)*F)). Contiguity matters:
    strided gathers measured at half HBM bandwidth (178 vs 342 GB/s).
    SWDGE (gpsimd) issues all input DMAs - it spreads across all 16 SDMA
    engines; the HWDGE rings only reach 8 of them.
  - VectorE folds the two channel halves (t[:, :F] + t[:, F:]) and rounds
    to fp32r for the PE (fp32r moving streams 1 cycle/row vs 4 for fp32).
  - TensorE reduces the remaining 128 channels (partition dim) per sample:
    stationary S_m [128, 8] is all-ones in column m = s%8 and zero
    elsewhere, so sample s lands in PSUM row m while other rows accumulate
    +0. Eight samples share one PSUM tile [8, F] (one accumulation group
    per 512-col bank chunk).
  - acc [8, F] -> SBUF via VectorE -> 64 KiB DMA out per group.
fp32r truncates the data mantissa (~1e-4 rel err), far below what could
flip a singular-value-positivity count (min sigma ~2e-2 here).
"""

import numpy as np

N_CORES = 8
NS = 16           # samples per core
C = 256           # channels
H, W = 32, 64
F = H * W         # 2048 spatial
CB = 2            # channel halves
P = 128           # partitions
SG = 8            # samples per PSUM group
NB = 4            # matmuls per sample (N=512 PSUM bank limit)
BN = F // NB      # 512

_CACHE = {}


def _build_nc():
    import concourse.bacc as bacc
    import concourse.mybir as mybir
    import concourse.tile as tile

    nc = bacc.Bacc(None, target_bir_lowering=False)
    f32 = mybir.dt.float32
    f32r = mybir.dt.float32r

    x_raw = nc.dram_tensor("x_raw", [NS, CB, P, F], f32, kind="ExternalInput")
    x_rect = nc.dram_tensor("x_rect", [NS, CB, P, F], f32, kind="ExternalInput")
    out_raw = nc.dram_tensor("out_raw", [NS, F], f32, kind="ExternalOutput")
    out_rect = nc.dram_tensor("out_rect", [NS, F], f32, kind="ExternalOutput")

    with tile.TileContext(nc) as tc:
        with (
            tc.tile_pool(name="io", bufs=6) as pool,
            tc.tile_pool(name="red", bufs=3) as redp,
            tc.tile_pool(name="small", bufs=2) as small,
            tc.tile_pool(name="psum", bufs=2, space="PSUM") as psum,
        ):
            # C[k, 8m + j] = 1 if j == m else 0; lhsT for sample s is the
            # [128, 8] slice C[:, 8m:8m+8] with m = s % 8.
            s_np = np.zeros((P, SG * SG), np.float32)
            for m in range(SG):
                s_np[:, SG * m + m] = 1.0
            s_dram = nc.inline_tensor(s_np, name="s_const")
            s_stage = small.tile([P, SG * SG], f32, tag="stat_stage")
            nc.sync.dma_start(s_stage[:], s_dram[:])
            S = small.tile([P, SG * SG], f32r, tag="stat")
            nc.vector.tensor_copy(S[:], s_stage[:])

            for xt, ot in ((x_raw, out_raw), (x_rect, out_rect)):
                for g in range(NS // SG):
                    acc = psum.tile([SG, F], f32, tag="acc")
                    for m in range(SG):
                        s = g * SG + m
                        # two contiguous 1 MiB transfers per sample
                        t0 = pool.tile([P, F], f32, tag="in0")
                        t1 = pool.tile([P, F], f32, tag="in1")
                        nc.gpsimd.dma_start(t0[:], xt[s, 0])
                        nc.gpsimd.dma_start(t1[:], xt[s, 1])
                        # fold channel halves + round to fp32r for the PE
                        tr = redp.tile([P, F], f32r, tag="red")
                        nc.vector.tensor_add(tr[:], t0[:], t1[:])
                        for j in range(NB):
                            nc.tensor.matmul(
                                acc[:, j * BN : (j + 1) * BN],
                                S[:, SG * m : SG * m + SG],
                                tr[:, j * BN : (j + 1) * BN],
                                start=(m == 0),
                                stop=(m == SG - 1),
                            )
                    osb = small.tile([SG, F], f32, tag="osb")
                    nc.vector.tensor_copy(osb[:], acc[:])
                    nc.sync.dma_start(ot[g * SG : (g + 1) * SG], osb[:])

    nc.compile()
    return nc


def _device_channel_sums(raw, rect, trace=False):
    """Run the bass kernel on 8 cores; return (sums_raw, sums_rect) [128, 2048]
    and the BassKernelResults."""
    from concourse.bass_utils import run_bass_kernel_spmd

    if "nc" not in _CACHE:
        _CACHE["nc"] = _build_nc()
    nc = _CACHE["nc"]

    raw5 = raw.reshape(N_CORES, NS, CB, P, F)
    rect5 = rect.reshape(N_CORES, NS, CB, P, F)
    in_maps = [{"x_raw": raw5[i], "x_rect": rect5[i]} for i in range(N_CORES)]
    res = run_bass_kernel_spmd(nc, in_maps, list(range(N_CORES)), trace=trace)

    sums_raw = np.concatenate([res.results[i]["out_raw"] for i in range(N_CORES)])
    sums_rect = np.concatenate([res.results[i]["out_rect"] for i in range(N_CORES)])
    return sums_raw, sums_rect, res


def _rank_from_sums(sums):
    # channel mean (exact: /256 is a power of two), normalize, svd, count
    m = (sums / np.float32(C)).astype(np.float32)
    nrm = np.linalg.norm(m, axis=1, keepdims=True)
    normed = (m / nrm).reshape(-1, H, W)
    s = np.linalg.svd(normed.astype(np.float32), compute_uv=False)
    return (s > 0.0).sum(axis=1).astype(np.float32)


def kernel(raw_feat, rectified_feat, trace=False):
    raw = np.ascontiguousarray(np.asarray(raw_feat, dtype=np.float32))
    rect = np.ascontiguousarray(np.asarray(rectified_feat, dtype=np.float32))

    sums_raw, sums_rect, res = _device_channel_sums(raw, rect, trace=trace)
    _CACHE["last_results"] = res
    _CACHE["last_sums"] = (sums_raw, sums_rect)

    rank1 = _rank_from_sums(sums_raw)
    rank2 = _rank_from_sums(sums_rect)
    loss = np.maximum(np.float32(0.0), -(rank1 - rank2))
    loss = loss.sum(dtype=np.float32) / np.float32(raw.shape[0])
    return np.asarray(loss, dtype=np.float32)
